# revision 1
# baseline (speedup 1.0000x reference)
"""BitLinear-1.58 (absmean ternary quantized linear) Trainium2 kernel.

Full-input contract: kernel(x[4,4096,4096] f32, weight[4096,4096] f32)
-> [4,4096,4096] f32, computing x @ Wq.T with
Wq = sign(W) * clip(round(|W|/gamma), 0, 1), gamma = mean(|W|) + 1e-6.

Sharding: data-parallel over tokens. Each of the 8 cores processes 2048
of the 16384 (b, s) rows with the full weight replicated; no collectives.

The scalar quantization threshold thr = gamma/2 is computed on the host
with the exact same jax-on-CPU op the reference uses (jnp.mean of |W|),
so the ternary decision boundary is bit-identical to the reference's;
knife-edge weights sit within one ulp of the threshold and would
otherwise flip. All O(N^3) compute and the full elementwise
quantization run on device.

Per-core pipeline (software-pipelined; emission order is per-engine
program order):
  - x loaded once, cast f32->f16 on ACT, transposed k-major on the PE
    (fp16 transpose-mode matmuls through an identity, PSUM->SBUF
    copyback) into a fully resident xT; no DRAM round-trip.
  - W quantized on DVE (q = (W > thr) - (W < -thr) in fp16), staged to
    DRAM, reloaded k-major per 256-column n-block with one XBAR
    transpose-DMA, double-buffered behind the previous block's matmuls.
  - Matmul: out[m128, n256] += xT[k128, m128].T @ WqT[k128, n256]
    accumulated over 32 k-tiles in PSUM (fp32), evicted via DVE copy.
"""

from contextlib import ExitStack

import numpy as np

import concourse.bass as bass
import concourse.mybir as mybir
import concourse.tile as tile
from concourse import bacc
from concourse.bass_utils import run_bass_kernel_spmd
from concourse.masks import make_identity

FP32 = mybir.dt.float32
FP16 = mybir.dt.float16

P = 128
EPS = 1e-6
N_CORES = 8

# Full-problem dims (hardcoded per harness contract)
B, S, D_IN, D_OUT = 4, 4096, 4096, 4096
M_FULL = B * S
M_LOC = M_FULL // N_CORES


def _bitlinear_body(ctx, tc, out_ap, x_ap, w_ap, thr_ap, nthr_ap,
                    M_loc, D_in, D_out, N_blk):
    nc = tc.nc
    KB = D_in // P              # k-tiles of 128
    NT = D_out // P             # weight row-tiles of 128
    KC = min(D_in, 1024)        # free-dim chunk for prep DMAs
    NCH = D_in // KC            # chunks per row-tile
    HK = min(D_in, 2048)        # x columns cast+transposed per group
    NHK = D_in // HK            # column groups per x row-tile
    KBH = HK // P               # k-tiles per column group
    MT = M_loc // P             # m-tiles
    MB = min(M_loc, 512)        # rows per xT sub-tile
    NMB = M_loc // MB           # xT sub-tiles
    MTB = MB // P               # m-tiles per xT sub-tile
    NB = D_out // N_blk         # n-blocks
    NBT = N_blk // P            # weight row-tiles per n-block

    dram = ctx.enter_context(tc.tile_pool(name="dram", bufs=1, space="DRAM"))
    wq16 = dram.tile([D_out, D_in], FP16)

    stats = ctx.enter_context(tc.tile_pool(name="stats", bufs=1, side="left"))
    thr_b = stats.tile([P, 1], FP32)
    nc.sync.dma_start(thr_b[:], thr_ap)
    nthr_b = stats.tile([P, 1], FP32)
    nc.sync.dma_start(nthr_b[:], nthr_ap)
    ident = stats.tile([P, P], FP16)
    make_identity(nc, ident[:])

    # prep pools cycle small tiles on the left; long-lived matmul-phase
    # tiles allocate from the right end so prep churn can't fragment them
    ld = ctx.enter_context(tc.tile_pool(name="ld", bufs=2, side="left"))
    q16 = ctx.enter_context(tc.tile_pool(name="q16", bufs=3, side="left"))
    xq16 = ctx.enter_context(tc.tile_pool(name="xq16", bufs=2, side="left"))
    cmp = ctx.enter_context(tc.tile_pool(name="cmp", bufs=1, side="left"))
    co = ctx.enter_context(tc.tile_pool(name="co", bufs=2, side="left"))
    xT = ctx.enter_context(tc.tile_pool(name="xT", bufs=NMB, side="right"))
    wqt = ctx.enter_context(tc.tile_pool(name="wqt", bufs=2, side="right"))
    ps = ctx.enter_context(tc.tile_pool(name="ps", bufs=4, space="PSUM"))
    tp = ctx.enter_context(tc.tile_pool(name="tp", bufs=4, space="PSUM"))

    def quant_chunk(nt, h):
        wt = ld.tile([P, KC], FP32, tag="ld")
        nc.sync.dma_start(wt[:], w_ap[nt * P:(nt + 1) * P, h * KC:(h + 1) * KC])
        a = cmp.tile([P, KC], FP16, tag="a")
        nc.vector.tensor_scalar(
            a[:], wt[:], thr_b[:], None, mybir.AluOpType.is_gt)
        bneg = cmp.tile([P, KC], FP16, tag="b")
        nc.vector.tensor_scalar(
            bneg[:], wt[:], nthr_b[:], None, mybir.AluOpType.is_lt)
        qt = q16.tile([P, KC], FP16, tag="q16")
        nc.vector.tensor_tensor(qt[:], a[:], bneg[:], mybir.AluOpType.subtract)
        nc.sync.dma_start(wq16[nt * P:(nt + 1) * P, h * KC:(h + 1) * KC], qt[:])

    def quant_w(nt):
        for h in range(NCH):
            quant_chunk(nt, h)

    xTts = [None] * NMB

    def xt_tile(mb):
        if xTts[mb] is None:
            xTts[mb] = xT.tile([P, KB, MB], FP16, tag="xT", name=f"xTt{mb}")
        return xTts[mb]

    def load_x(mt):
        # load+cast one x row-tile, transpose k-major on the PE into xT
        t = xt_tile(mt // MTB)
        mc = (mt % MTB) * P
        for g in range(NHK):
            xq = xq16.tile([P, HK], FP16, tag="xq")
            for h in range(HK // KC):
                c = g * HK + h * KC
                xt_ = ld.tile([P, KC], FP32, tag="ld")
                nc.sync.dma_start(xt_[:], x_ap[mt * P:(mt + 1) * P, c:c + KC])
                nc.scalar.activation(
                    xq[:, h * KC:(h + 1) * KC], xt_[:],
                    mybir.ActivationFunctionType.Copy)
            for j in range(KBH):
                pt = tp.tile([P, P], FP16)
                nc.tensor.transpose(pt[:], xq[:, j * P:(j + 1) * P], ident[:])
                # alternate copyback engine: ACT also runs the casts
                eng = nc.vector if j % 2 == 0 else nc.scalar
                if eng is nc.vector:
                    eng.tensor_copy(
                        out=t[:, g * KBH + j, mc:mc + P], in_=pt[:])
                else:
                    nc.scalar.activation(
                        t[:, g * KBH + j, mc:mc + P], pt[:],
                        mybir.ActivationFunctionType.Copy)

    def matmuls(nb, wq_t, mts):
        for mt in mts:
            xTt = xTts[mt // MTB]
            mc = (mt % MTB) * P
            pst = ps.tile([P, N_blk], FP32)
            for kb in range(KB):
                nc.tensor.matmul(
                    pst[:],
                    xTt[:, kb, mc:mc + P],
                    wq_t[:, kb, :],
                    start=(kb == 0),
                    stop=(kb == KB - 1),
                )
            cot = co.tile([P, N_blk], FP32)
            nc.vector.tensor_copy(out=cot[:], in_=pst[:])
            nc.sync.dma_start(
                out_ap[mt * P:(mt + 1) * P, nb * N_blk:(nb + 1) * N_blk],
                cot[:],
            )

    def wqt_load(nb):
        wq_t = wqt.tile([P, KB, N_blk], FP16, tag="wq_t")
        nc.sync.dma_start_transpose(
            wq_t[:], wq16[nb * N_blk:(nb + 1) * N_blk, :])
        return wq_t

    # startup: quantize n-blocks 0..1 and interleave x ingestion with
    # their matmuls one 512-row group at a time, so the PE has enough
    # work to cover the ingest stream
    quant_done = set()
    second = 1 < NB
    # background quantize chunks for n-blocks 1..2, pumped between x
    # tile loads so neither the PE's x feed nor the weight feed starves
    bg = [(nt, h)
          for nt in range(NBT, min(3 * NBT, NT))
          for h in range(NCH)] if second else []
    bgpos = [0]

    def pump(n):
        while n > 0 and bgpos[0] < len(bg):
            nt, h = bg[bgpos[0]]
            quant_chunk(nt, h)
            bgpos[0] += 1
            n -= 1

    nb1_chunks = NBT * NCH if second else 0

    # first x rows ahead of the weight stream: PE transposes start early
    load_x(0)
    for nt in range(NBT):
        quant_w(nt)
    wq_t0 = wqt_load(0)
    matmuls(0, wq_t0, [0])
    for mt in range(1, MTB):
        load_x(mt)
        matmuls(0, wq_t0, [mt])

    wq_t1 = None
    for mb in range(1, NMB):
        for mt in range(mb * MTB, (mb + 1) * MTB):
            pump(2)
            load_x(mt)
            matmuls(0, wq_t0, [mt])
            if wq_t1 is not None:
                matmuls(1, wq_t1, [mt])
        if second and wq_t1 is None:
            pump(nb1_chunks - bgpos[0])  # ensure n-block 1 fully staged
            wq_t1 = wqt_load(1)
            matmuls(1, wq_t1, range((mb + 1) * MTB))
    if second and wq_t1 is None:
        pump(nb1_chunks - bgpos[0])
        wq_t1 = wqt_load(1)
        matmuls(1, wq_t1, range(MT))
    pump(len(bg))  # drain remaining background chunks (n-block 2)
    if len(bg) > nb1_chunks:
        quant_done.add(2)

    for nb in range(2, NB):
        if nb not in quant_done:
            for nt in range(nb * NBT, (nb + 1) * NBT):
                quant_w(nt)
        wq_t = wqt_load(nb)
        matmuls(nb, wq_t, range(MT))


def build_nc(M_loc=M_LOC, D_in=D_IN, D_out=D_OUT, N_blk=256):
    nc = bacc.Bacc("TRN2", target_bir_lowering=False, debug=False,
                   num_devices=N_CORES)
    x = nc.dram_tensor("x", [M_loc, D_in], FP32, kind="ExternalInput").ap()
    w = nc.dram_tensor("w", [D_out, D_in], FP32, kind="ExternalInput").ap()
    thr = nc.dram_tensor("thr", [P, 1], FP32, kind="ExternalInput").ap()
    nthr = nc.dram_tensor("nthr", [P, 1], FP32, kind="ExternalInput").ap()
    out = nc.dram_tensor("out", [M_loc, D_out], FP32, kind="ExternalOutput").ap()
    with tile.TileContext(nc) as tc:
        with ExitStack() as ctx:
            _bitlinear_body(ctx, tc, out, x, w, thr, nthr,
                            M_loc, D_in, D_out, N_blk)
    nc.compile()
    return nc


_NC = None


def _get_nc():
    global _NC
    if _NC is None:
        _NC = build_nc()
    return _NC


def _host_threshold(weight: np.ndarray) -> np.float32:
    """gamma/2 with gamma bit-identical to the reference's jax-on-CPU mean."""
    import jax
    import jax.numpy as jnp

    cpu = jax.devices("cpu")[0]
    with jax.default_device(cpu):
        gamma = jnp.mean(jnp.abs(jnp.asarray(weight, dtype=jnp.float32)))
    gamma = np.float32(gamma) + np.float32(EPS)
    return np.float32(gamma * np.float32(0.5))


def kernel(x: np.ndarray, weight: np.ndarray, **_ignored) -> np.ndarray:
    assert x.shape == (B, S, D_IN) and weight.shape == (D_OUT, D_IN)
    xf = np.ascontiguousarray(x.reshape(M_FULL, D_IN).astype(np.float32, copy=False))
    w = np.ascontiguousarray(weight.astype(np.float32, copy=False))
    thr = _host_threshold(w)
    thr_arr = np.full((P, 1), thr, dtype=np.float32)
    nthr_arr = -thr_arr
    nc = _get_nc()
    in_maps = [
        {"x": np.ascontiguousarray(xf[i * M_LOC:(i + 1) * M_LOC]), "w": w,
         "thr": thr_arr, "nthr": nthr_arr}
        for i in range(N_CORES)
    ]
    res = run_bass_kernel_spmd(nc, in_maps, core_ids=list(range(N_CORES)))
    outs = [res.results[i]["out"] for i in range(N_CORES)]
    full = np.concatenate(outs, axis=0)
    if not np.isfinite(full).all():
        # cold-start transient guard: retry once
        res = run_bass_kernel_spmd(nc, in_maps, core_ids=list(range(N_CORES)))
        outs = [res.results[i]["out"] for i in range(N_CORES)]
        full = np.concatenate(outs, axis=0)
    return full.reshape(B, S, D_OUT).astype(np.float32, copy=False)


if __name__ == "__main__":
    # quick smoke on small shapes via CoreSim
    from concourse.bass_interp import CoreSim

    M_loc, D_in, D_out = 256, 512, 512
    nc = build_nc(M_loc=M_loc, D_in=D_in, D_out=D_out, N_blk=256)
    rng = np.random.default_rng(0)
    xs = rng.standard_normal((M_loc, D_in), dtype=np.float32)
    ws = rng.standard_normal((D_out, D_in), dtype=np.float32)
    gamma = np.abs(ws).mean(dtype=np.float32) + np.float32(EPS)
    thr = np.float32(gamma * np.float32(0.5))
    sim = CoreSim(nc, require_finite=True, require_nnan=True)
    sim.tensor("x")[:] = xs
    sim.tensor("w")[:] = ws
    sim.tensor("thr")[:] = np.full((P, 1), thr, np.float32)
    sim.tensor("nthr")[:] = np.full((P, 1), -thr, np.float32)
    sim.simulate(check_with_hw=False)
    got = np.array(sim.tensor("out"))

    wq = np.sign(ws) * np.clip(np.round(np.abs(ws / gamma)), None, 1.0)
    exp = xs @ wq.T.astype(np.float32)
    err = np.abs(got - exp).max() / np.abs(exp).max()
    print("sim rel err:", err)



# revision 11
# speedup vs baseline: 1.7912x; 1.7912x over previous
"""BitLinear-1.58 (absmean ternary quantized linear) Trainium2 kernel.

Full-input contract: kernel(x[4,4096,4096] f32, weight[4096,4096] f32)
-> [4,4096,4096] f32, computing x @ Wq.T with
Wq = sign(W) * clip(round(|W|/gamma), 0, 1), gamma = mean(|W|) + 1e-6.

Sharding: data-parallel over tokens. Each of the 8 cores processes 2048
of the 16384 (b, s) rows with the full weight replicated; no collectives.

The scalar quantization threshold thr = gamma/2 is computed on the host
with the exact same jax-on-CPU op the reference uses (jnp.mean of |W|),
so the ternary decision boundary is bit-identical to the reference's.
x is pre-cast to f16 on the host (rel rounding 2^-11; output error well
under 1e-3 of scale) to halve its DMA volume.

Per-core pipeline, built around fp8 DoubleRow matmuls (2 k-tiles per
instruction, 0.5 PE cycles per output column — 4x the f16 rate):
  - x (f16) is split into hi = fp8e4(x), lo = fp8e4(x - hi); the matmul
    accumulates hi@WqT + lo@WqT over an effective contraction of 8192,
    recovering ~2^-8 relative precision on x. Wq in {-1,0,+1} is exact
    in fp8e4.
  - x ingest: PE transpose (f16 through an identity) into PSUM, then
    ACT casts hi (PSUM f16 -> SBUF fp8) and DVE/gpsimd subtract lo.
    Both k-major operands stay resident in SBUF (64KB/partition each).
  - W quantize, streamed per 256-column n-block, all on-chip (no DRAM
    round-trip): gpsimd a = (w < -thr); DVE q16 = (w > thr) - a;
    PE transpose; ACT copyback-cast PSUM f16 -> SBUF fp8, k-major.
  - Matmuls: out[m128, n256] += xT8[k128, 2, m128].T @ wqT8[k128, 2,
    n256] (DoubleRow), 32 instructions per output tile. PSUM f32 is
    evicted to f16 (engine rotated) and DMA'd out; the host upcasts.
"""

from contextlib import ExitStack

import numpy as np

import concourse.bass as bass
import concourse.mybir as mybir
import concourse.tile as tile
from concourse import bacc
from concourse.bass_utils import run_bass_kernel_spmd
from concourse.masks import make_identity

FP32 = mybir.dt.float32
FP16 = mybir.dt.float16
FP8 = mybir.dt.float8e4

P = 128
EPS = 1e-6
N_CORES = 8

# Full-problem dims (hardcoded per harness contract)
B, S, D_IN, D_OUT = 4, 4096, 4096, 4096
M_FULL = B * S
M_LOC = M_FULL // N_CORES

DR = mybir.MatmulPerfMode.DoubleRow
COPY = mybir.ActivationFunctionType.Copy


def _bitlinear_body(ctx, tc, out_ap, x_ap, w_ap, thr, M_loc, D_in, D_out,
                    N_blk):
    nc = tc.nc
    KB = D_in // P              # k-tiles of 128
    MT = M_loc // P             # m-tiles
    NB = D_out // N_blk         # n-blocks streamed
    RT = N_blk // P             # weight row-tiles per n-block
    KG = min(8, KB)             # k-tiles per PSUM transpose-staging group
    NKG = KB // KG              # staging groups per row-tile
    CW = min(2048, D_in)        # load/elementwise chunk width (free dim)
    NCH = D_in // CW            # chunks per row
    GPC = CW // (KG * P)        # staging groups per chunk

    stats = ctx.enter_context(tc.tile_pool(name="stats", bufs=1, side="left"))
    ident = stats.tile([P, P], FP16)
    make_identity(nc, ident[:])

    xld = ctx.enter_context(tc.tile_pool(name="xld", bufs=2, side="left"))
    wld = ctx.enter_context(tc.tile_pool(name="wld", bufs=2, side="left"))
    apool = ctx.enter_context(tc.tile_pool(name="apool", bufs=2, side="left"))
    q16p = ctx.enter_context(tc.tile_pool(name="q16", bufs=3 * NCH, side="left"))
    co = ctx.enter_context(tc.tile_pool(name="co", bufs=6, side="left"))
    wq_pool = ctx.enter_context(tc.tile_pool(name="wq", bufs=2, side="right"))
    xT = ctx.enter_context(tc.tile_pool(name="xT", bufs=1, side="right"))
    ps = ctx.enter_context(tc.tile_pool(name="ps", bufs=2, space="PSUM"))
    tp = ctx.enter_context(tc.tile_pool(name="tp", bufs=2, space="PSUM"))
    tpx = ctx.enter_context(tc.tile_pool(name="tpx", bufs=2, space="PSUM"))

    xhi = xT.tile([P, KB, M_loc], FP8, name="xhi")
    xlo = xT.tile([P, KB, M_loc], FP8, name="xlo")

    # ---- W quantize: IO/elementwise part (DMA + gpsimd + DVE) ----
    q16s = {}

    def quant_io(nb, r):
        n0 = nb * N_blk + r * P
        for h in range(NCH):
            wt = wld.tile([P, CW], FP32, tag="wt")
            nc.sync.dma_start(wt[:], w_ap[n0:n0 + P, h * CW:(h + 1) * CW])
            a = apool.tile([P, CW], FP16, tag="a")
            nc.gpsimd.tensor_scalar(a[:], wt[:], -thr, None,
                                    mybir.AluOpType.is_lt)
            q16 = q16p.tile([P, CW], FP16, tag="q16")
            nc.vector.scalar_tensor_tensor(
                q16[:], wt[:], thr, a[:],
                mybir.AluOpType.is_gt, mybir.AluOpType.subtract)
            q16s[(nb, r, h)] = q16

    # ---- W quantize: transpose part (PE + ACT copyback-cast) ----
    wqs = {}

    def quant_T_group(nb, idx):
        # idx in [0, RT*NKG): one group of KG transposes + one copyback
        if (nb, 0) not in wqs:
            wqs[nb] = wq_pool.tile([P, KB, N_blk], FP8, tag="wq",
                                   name=f"wq{nb % 2}")
            wqs[(nb, 0)] = True
        wq = wqs[nb]
        r, g = divmod(idx, NKG)
        h, gl = divmod(g, GPC)
        q16 = q16s[(nb, r, h)]
        tpt = tp.tile([P, KG * P], FP16, tag="wtp")
        for i in range(KG):
            k = gl * KG + i
            nc.tensor.transpose(tpt[:, i * P:(i + 1) * P],
                                q16[:, k * P:(k + 1) * P], ident[:])
        nc.scalar.activation(wq[:, g * KG:(g + 1) * KG, r * P:(r + 1) * P],
                             tpt[:], COPY)

    def quant_T(nb):
        for idx in range(RT * NKG):
            quant_T_group(nb, idx)

    # ---- x ingest: one m-tile (DMA + PE-T + hi/lo split) ----
    def load_x(mt):
        mc = mt * P
        for h in range(NCH):
            xq = xld.tile([P, CW], FP16, tag="xq")
            nc.sync.dma_start(xq[:],
                              x_ap[mt * P:(mt + 1) * P, h * CW:(h + 1) * CW])
            for gl in range(GPC):
                g = h * GPC + gl
                tpt = tpx.tile([P, KG * P], FP16, tag="xtp")
                for i in range(KG):
                    k = gl * KG + i
                    nc.tensor.transpose(tpt[:, i * P:(i + 1) * P],
                                        xq[:, k * P:(k + 1) * P], ident[:])
                hi_t = xhi[:, g * KG:(g + 1) * KG, mc:mc + P]
                nc.scalar.activation(hi_t, tpt[:], COPY)
                # gpsimd cannot touch PSUM; lo stays on DVE
                nc.vector.tensor_tensor(xlo[:, g * KG:(g + 1) * KG, mc:mc + P],
                                        tpt[:], hi_t, mybir.AluOpType.subtract)

    # ---- matmuls for one (nb, mt): 32 DoubleRow + evict + store ----
    def evict(i, cot, pst):
        # gpsimd cannot touch PSUM; alternate the two engines that can
        if i % 2 == 0:
            nc.scalar.activation(cot[:], pst[:], COPY)
        else:
            nc.vector.tensor_copy(out=cot[:], in_=pst[:])

    def matmuls(nb, mt):
        wq = wqs[nb]
        mc = mt * P
        pst = ps.tile([P, N_blk], FP32, tag="pst")
        nsteps = KB // 2
        for src_i, src in enumerate((xhi, xlo)):
            for j in range(nsteps):
                nc.tensor.matmul(
                    pst[:],
                    src[:, 2 * j:2 * j + 2, mc:mc + P],
                    wq[:, 2 * j:2 * j + 2, :],
                    start=(src_i == 0 and j == 0),
                    stop=(src_i == 1 and j == nsteps - 1),
                    perf_mode=DR,
                )
        cot = co.tile([P, N_blk], FP16, tag="cot")
        evict(nb * MT + mt, cot, pst)
        nc.sync.dma_start(
            out_ap[mt * P:(mt + 1) * P, nb * N_blk:(nb + 1) * N_blk], cot[:])

    # ---- emission schedule ----
    # Block 0 quantize fully up front; x ingest interleaves with block-0
    # matmuls; later blocks' transposes spread across the previous block's
    # matmul stream so wq double-buffering hides quantize latency.
    ngroups = RT * NKG

    def quant_T_frac(nb, mt):
        # spread ngroups over MT m-tiles
        lo = ngroups * mt // MT
        hi = ngroups * (mt + 1) // MT
        for idx in range(lo, hi):
            quant_T_group(nb, idx)

    for r in range(RT):
        quant_io(0, r)
    quant_T(0)
    if NB > 1:
        for r in range(RT):
            quant_io(1, r)
    for mt in range(MT):
        load_x(mt)
        matmuls(0, mt)
        if NB > 1:
            quant_T_frac(1, mt)
    for nb in range(1, NB):
        if nb + 1 < NB:
            for r in range(RT):
                quant_io(nb + 1, r)
        for mt in range(MT):
            matmuls(nb, mt)
            if nb + 1 < NB:
                quant_T_frac(nb + 1, mt)


def build_nc(M_loc=M_LOC, D_in=D_IN, D_out=D_OUT, N_blk=256, thr=0.5):
    nc = bacc.Bacc("TRN2", target_bir_lowering=False, debug=False,
                   num_devices=N_CORES)
    x = nc.dram_tensor("x", [M_loc, D_in], FP16, kind="ExternalInput").ap()
    w = nc.dram_tensor("w", [D_out, D_in], FP32, kind="ExternalInput").ap()
    out = nc.dram_tensor("out", [M_loc, D_out], FP16, kind="ExternalOutput").ap()
    with tile.TileContext(nc) as tc:
        with ExitStack() as ctx:
            _bitlinear_body(ctx, tc, out, x, w, thr, M_loc, D_in, D_out, N_blk)
    nc.compile()
    return nc


_NC = None
_NC_THR = None


def _get_nc(thr):
    global _NC, _NC_THR
    if _NC is None or _NC_THR != thr:
        _NC = build_nc(thr=thr)
        _NC_THR = thr
    return _NC


def _host_threshold(weight: np.ndarray) -> float:
    """gamma/2 with gamma bit-identical to the reference's jax-on-CPU mean."""
    import jax
    import jax.numpy as jnp

    cpu = jax.devices("cpu")[0]
    with jax.default_device(cpu):
        gamma = jnp.mean(jnp.abs(jnp.asarray(weight, dtype=jnp.float32)))
    gamma = np.float32(gamma) + np.float32(EPS)
    return float(np.float32(gamma * np.float32(0.5)))


def kernel(x: np.ndarray, weight: np.ndarray, **_ignored) -> np.ndarray:
    assert x.shape == (B, S, D_IN) and weight.shape == (D_OUT, D_IN)
    xf = x.reshape(M_FULL, D_IN).astype(np.float16)
    w = np.ascontiguousarray(weight.astype(np.float32, copy=False))
    thr = _host_threshold(w)
    nc = _get_nc(thr)
    in_maps = [
        {"x": np.ascontiguousarray(xf[i * M_LOC:(i + 1) * M_LOC]), "w": w}
        for i in range(N_CORES)
    ]
    res = run_bass_kernel_spmd(nc, in_maps, core_ids=list(range(N_CORES)))
    outs = [res.results[i]["out"] for i in range(N_CORES)]
    full = np.concatenate(outs, axis=0)
    if not np.isfinite(full.astype(np.float32)).all():
        # cold-start transient guard: retry once
        res = run_bass_kernel_spmd(nc, in_maps, core_ids=list(range(N_CORES)))
        outs = [res.results[i]["out"] for i in range(N_CORES)]
        full = np.concatenate(outs, axis=0)
    return full.reshape(B, S, D_OUT).astype(np.float32)


if __name__ == "__main__":
    # quick smoke on small shapes via CoreSim
    import ml_dtypes
    from concourse.bass_interp import CoreSim

    M_loc, D_in, D_out = 256, 1024, 512
    rng = np.random.default_rng(0)
    xs = rng.standard_normal((M_loc, D_in)).astype(np.float16)
    ws = rng.standard_normal((D_out, D_in)).astype(np.float32)
    gamma = np.abs(ws).mean(dtype=np.float32) + np.float32(EPS)
    thr = float(np.float32(gamma * np.float32(0.5)))
    nc = build_nc(M_loc=M_loc, D_in=D_in, D_out=D_out, N_blk=256, thr=thr)
    sim = CoreSim(nc, require_finite=True, require_nnan=True)
    sim.tensor("x")[:] = xs
    sim.tensor("w")[:] = ws
    sim.simulate(check_with_hw=False)
    got = np.array(sim.tensor("out")).astype(np.float32)

    wq = (np.where(ws > thr, 1.0, 0.0)
          - np.where(ws < -thr, 1.0, 0.0)).astype(np.float32)
    xh = xs.astype(ml_dtypes.float8_e4m3fn).astype(np.float32)
    xl = (xs.astype(np.float32) - xh).astype(ml_dtypes.float8_e4m3fn)
    exp = (xh + xl.astype(np.float32)) @ wq.T
    exact = xs.astype(np.float32) @ wq.T
    print("sim err vs fp8 model:", np.abs(got - exp).max())
    print("sim rel err vs exact:",
          np.abs(got - exact).max() / np.abs(exact).max())
